# revision 16
# baseline (speedup 1.0000x reference)
"""Trainium2 Bass kernel for MindSpeed TE grouped linear (MoE grouped GEMM).

Computes, for E=64 experts with row splits m_splits (sum = 32768):
    y[rows_e, :] = x[rows_e, :] @ W[e].T        W[e]: [1408, 2048]

Strategy: pure expert-parallel over 8 NeuronCores — core c owns experts
[8c, 8c+8) and their (contiguous) token rows. No collectives; gather is a
host-side concat. Both operands are cast to fp16 on the host: the PE runs
fp16 at full rate (78.6 TF/s) while halving HBM traffic, which moves the
kernel from DMA-bound (149 MB/core, ~416 us) to compute-bound (~300 us).
y is written back as fp16 too (host upcasts); measured rel err ~3e-4.

Layout: host pre-packs both operands so every DMA is a contiguous run per
partition (128 descriptors, no strided row descriptors — descriptor
generation was ~3.5 us per 1 MB x chunk in the naive [P, KO, T] layout):
  xT[p, KO*t0c + ko*csz + j]  = x[t0c + j, ko*128 + p]   per token-chunk
  wT[p, woff(e,nt,q) + k*nsz + j] = W[e][n0+j, (q*KQ+k)*128 + p]
DMA queues are split per operand (x: sync, W: scalar, y: gpsimd) so the
first matmul's dependencies never serialize behind later traffic, and a
burst of scratch matmuls at kernel start warms the PE HAM clock gate
(1.2 -> 2.4 GHz) during the initial DMA wait.
"""

import math

import numpy as np

import concourse.mybir as mybir
import concourse.tile as tile
from concourse import bacc
from concourse.bass_utils import run_bass_kernel_spmd

N_CORES = 8
P = 128
IN_SIZE = 2048
OUT_SIZE = 1408
KO = IN_SIZE // P  # 16 contraction subtiles
N_TILE = 512
KQ = 4  # W arrives in quarter-K chunks for fine pipelining
NQ = KO // KQ
WARMUP_MM = 40  # scratch matmuls: warm the HAM clock gate AND absorb the
# startup DMA ramp (W-quarter supply is HBM-limited until prefetch builds)

_nc_cache: dict = {}


def _n_tiles():
    tiles = []
    n0 = 0
    while n0 < OUT_SIZE:
        nsz = min(N_TILE, OUT_SIZE - n0)
        tiles.append((n0, nsz))
        n0 += nsz
    return tiles


SEG_MAX = 6 * P  # per-segment token cap: bounds SBUF for arbitrary splits


def _plan_layout(pattern: tuple):
    """Pure function of the (padded) per-expert token pattern; used by both
    _build (device program) and kernel() (host repack) so the DRAM layouts
    agree. Returns segments, processing order, x chunking and W block
    offsets (elements, per partition)."""
    T = sum(pattern)
    segs = []  # (expert, token offset, token count <= SEG_MAX)
    t = 0
    for e, m in enumerate(pattern):
        s0 = 0
        while s0 < m:
            sm = min(SEG_MAX, m - s0)
            segs.append((e, t + s0, sm))
            s0 += sm
        t += m
    chunks = [-(-s[2] // (2 * P)) for s in segs]
    # Fast config (tuned for the alternating small/big pattern): x in
    # 2-m-tile granules, larger segment of each pair first. Requires any
    # adjacent pair to fit in 4 x-slots (bufs=5 leaves one for prefetch);
    # otherwise whole-segment x tiles, which always fit with bufs=2.
    fast = len(segs) > 0 and all(
        chunks[i] + chunks[i + 1] <= 4 for i in range(0, len(segs) - 1, 2)
    )
    if fast:
        XC = 2 * P
        x_bufs = 5
        order = []
        for i in range(0, len(segs) - 1, 2):
            a, b = i, i + 1
            order.extend([b, a] if segs[b][2] > segs[a][2] else [a, b])
        if len(segs) % 2:
            order.append(len(segs) - 1)
        if len(order) >= 2:
            # Small segment of the first pair first: fewer x bytes on the
            # critical path, so the first W quarters get full HBM bandwidth.
            order[0:2] = sorted(order[0:2], key=lambda s: segs[s][2])
    else:
        XC = SEG_MAX
        x_bufs = 2
        order = list(range(len(segs)))

    # x chunks in token order; chunk starting at token t0c occupies
    # xT[:, KO*t0c : KO*(t0c+csz)] (contiguous per partition).
    x_chunks = []  # (t0c, csz)
    seg_chunks = []  # per segment: list of chunk indices
    for e, t0, m in segs:
        idxs = []
        c0 = 0
        while c0 < m:
            csz = min(XC, m - c0)
            idxs.append(len(x_chunks))
            x_chunks.append((t0 + c0, csz))
            c0 += csz
        seg_chunks.append(idxs)

    # W blocks in canonical (expert, n-tile, quarter) order.
    w_offs = {}
    off = 0
    E_loc = len(pattern)
    for e in range(E_loc):
        for nt, (n0, nsz) in enumerate(_n_tiles()):
            for q in range(NQ):
                w_offs[(e, nt, q)] = off
                off += KQ * nsz
    return {
        "T": T,
        "segs": segs,
        "order": order,
        "fast": fast,
        "XC": XC,
        "x_bufs": x_bufs,
        "x_chunks": x_chunks,
        "seg_chunks": seg_chunks,
        "w_offs": w_offs,
        "w_total": off,
    }


def _build(pattern: tuple) -> "bacc.Bacc":
    """One SPMD program: `pattern` = per-expert (padded) token counts for the
    8 local experts of a core; identical across cores."""
    pl = _plan_layout(pattern)
    T, XC = pl["T"], pl["XC"]
    nc = bacc.Bacc(None, target_bir_lowering=False, name="grouped_linear")
    x_dt = mybir.dt.float16
    w_dt = mybir.dt.float16
    y_dt = mybir.dt.float16
    xT = nc.dram_tensor("xT", [P, KO * T], x_dt, kind="ExternalInput")
    wT = nc.dram_tensor("wT", [P, pl["w_total"]], w_dt, kind="ExternalInput")
    y = nc.dram_tensor("y", [T, OUT_SIZE], y_dt, kind="ExternalOutput")

    with tile.TileContext(nc) as tc:
        with (
            tc.tile_pool(name="xp", bufs=pl["x_bufs"]) as xpool,
            tc.tile_pool(name="wp", bufs=10) as wpool,
            tc.tile_pool(name="op", bufs=8) as opool,
            tc.tile_pool(name="ps", bufs=6, space="PSUM") as pspool,
            tc.tile_pool(name="wu", bufs=1) as wupool,
            tc.tile_pool(name="wq", bufs=1, space="PSUM") as wqpool,
        ):
            # HAM warm-up: scratch matmuls with no DMA dependencies run
            # during the initial x/W load, lifting the PE to 2.4 GHz
            # before the first real matmul.
            wu_a = wupool.tile([P, P], x_dt, tag="wua", name="wu_a")
            wu_b = wupool.tile([P, N_TILE], w_dt, tag="wub", name="wu_b")
            wu_ps = wqpool.tile([P, N_TILE], mybir.dt.float32, tag="wups", name="wu_ps")
            nc.vector.memset(wu_a, 0)
            nc.vector.memset(wu_b, 0)
            for _ in range(WARMUP_MM):
                nc.tensor.matmul(wu_ps, wu_a, wu_b, start=True, stop=True)

            for si in pl["order"]:
                e, t0, m = pl["segs"][si]
                mts = m // P
                x_cs = []
                for ci in pl["seg_chunks"][si]:
                    t0c, csz = pl["x_chunks"][ci]
                    x_c = xpool.tile([P, KO * XC], x_dt, tag="x", name="x_c")
                    nc.sync.dma_start(
                        x_c[:, : KO * csz], xT[:, KO * t0c : KO * (t0c + csz)]
                    )
                    x_cs.append((x_c, t0c, csz))
                o_ts = [
                    opool.tile([P, OUT_SIZE], y_dt, tag="o", name="o_t")
                    for _ in range(mts)
                ]
                n_tiles = _n_tiles()
                for nt, (n0, nsz) in enumerate(n_tiles):
                    w_qs = []
                    for q in range(NQ):
                        w_q = wpool.tile([P, KQ * N_TILE], w_dt, tag="w", name="w_q")
                        woff = pl["w_offs"][(e, nt, q)]
                        nc.scalar.dma_start(
                            w_q[:, : KQ * nsz], wT[:, woff : woff + KQ * nsz]
                        )
                        w_qs.append(w_q)
                    for mt in range(mts):
                        x_c, t0c, csz = x_cs[(mt * P) // XC]
                        xoff = t0 + mt * P - t0c
                        ps_t = pspool.tile(
                            [P, N_TILE], mybir.dt.float32, tag="ps", name="ps_t"
                        )
                        for q in range(NQ):
                            for k in range(KQ):
                                ko = q * KQ + k
                                nc.tensor.matmul(
                                    ps_t[:, :nsz],
                                    x_c[:, ko * csz + xoff : ko * csz + xoff + P],
                                    w_qs[q][:, k * nsz : (k + 1) * nsz],
                                    start=(ko == 0),
                                    stop=(ko == KO - 1),
                                )
                        nc.vector.tensor_copy(
                            o_ts[mt][:, n0 : n0 + nsz], ps_t[:, :nsz]
                        )
                        if nt == len(n_tiles) - 1:
                            # full output rows for this m-tile are ready:
                            # one fully-contiguous 128x1408 store
                            nc.sync.dma_start(
                                y[t0 + mt * P : t0 + (mt + 1) * P, :], o_ts[mt]
                            )
    nc.compile()
    return nc


def _get_nc(pattern: tuple) -> "bacc.Bacc":
    nc = _nc_cache.get(pattern)
    if nc is None:
        nc = _build(pattern)
        _nc_cache[pattern] = nc
    return nc


def _plan(splits: np.ndarray):
    """Choose a per-core expert-size pattern (identical across cores, sizes
    multiples of 128). Returns (padded_pattern, per-core list of per-expert
    actual sizes)."""
    E = len(splits)
    epc = E // N_CORES
    per_core = [tuple(int(s) for s in splits[c * epc : (c + 1) * epc]) for c in range(N_CORES)]
    uniform = all(p == per_core[0] for p in per_core)
    if uniform:
        padded = tuple(128 * math.ceil(s / 128) for s in per_core[0])
    else:
        m_pad = 128 * math.ceil(int(max(splits.max(), 1)) / 128)
        padded = (m_pad,) * epc
    return padded, per_core


def kernel(x: np.ndarray, W: np.ndarray, m_splits: np.ndarray, _profile=None) -> np.ndarray:
    x = np.ascontiguousarray(np.asarray(x), dtype=np.float32)
    W = np.ascontiguousarray(np.asarray(W), dtype=np.float32)
    raw = np.asarray(m_splits).astype(np.int64)
    E = raw.shape[0]
    assert E % N_CORES == 0 and W.shape[0] == E
    epc = E // N_CORES
    # Mirror the reference's python-slice semantics: x[offs[e]:offs[e+1]]
    # clips to the array bounds, so effective sizes come from clipped offsets.
    raw_offs = np.concatenate([[0], np.cumsum(np.maximum(raw, 0))])
    lo = np.minimum(raw_offs[:-1], x.shape[0])
    hi = np.minimum(raw_offs[1:], x.shape[0])
    splits = np.maximum(hi - lo, 0)
    offs = np.concatenate([[0], np.cumsum(splits)])
    total = int(offs[-1])

    padded, per_core = _plan(splits)
    pofs = np.concatenate([[0], np.cumsum(padded)])
    T_pad = int(pofs[-1])

    nc = _get_nc(padded)
    pl = _plan_layout(padded)

    n_tiles = _n_tiles()
    in_maps = []
    for c in range(N_CORES):
        if tuple(padded) == per_core[c]:
            xs = x[lo[c * epc] : hi[(c + 1) * epc - 1]]
        else:
            xs = np.zeros((T_pad, IN_SIZE), dtype=np.float32)
            for e in range(epc):
                g = c * epc + e
                xs[pofs[e] : pofs[e] + splits[g]] = x[lo[g] : hi[g]]
        xs16 = xs.astype(np.float16)
        xT = np.empty((P, KO * T_pad), dtype=np.float16)
        for t0c, csz in pl["x_chunks"]:
            blk = xs16[t0c : t0c + csz].reshape(csz, KO, P).transpose(2, 1, 0)
            xT[:, KO * t0c : KO * (t0c + csz)] = blk.reshape(P, KO * csz)
        wfull = (
            W[c * epc : (c + 1) * epc]
            .reshape(epc, OUT_SIZE, KO, P)
            .transpose(0, 3, 2, 1)
            .astype(np.float16)
        )  # [epc, P, KO, OUT]
        wT = np.empty((P, pl["w_total"]), dtype=np.float16)
        for (e, nt, q), woff in pl["w_offs"].items():
            n0, nsz = n_tiles[nt]
            blk = wfull[e, :, q * KQ : (q + 1) * KQ, n0 : n0 + nsz]
            wT[:, woff : woff + KQ * nsz] = blk.reshape(P, KQ * nsz)
        in_maps.append({"xT": np.ascontiguousarray(xT), "wT": np.ascontiguousarray(wT)})

    kwargs = dict(_profile) if _profile else {}
    res = run_bass_kernel_spmd(nc, in_maps, core_ids=list(range(N_CORES)), **kwargs)
    if _profile is not None:
        _profile["result"] = res

    out = np.empty((total, OUT_SIZE), dtype=np.float32)
    for c in range(N_CORES):
        yc = res.results[c]["y"]
        for e in range(epc):
            g = c * epc + e
            out[offs[g] : offs[g + 1]] = yc[pofs[e] : pofs[e] + splits[g]]
    return out


# revision 19
# speedup vs baseline: 1.0699x; 1.0699x over previous
"""Trainium2 Bass kernel for MindSpeed TE grouped linear (MoE grouped GEMM).

Computes, for E=64 experts with row splits m_splits (sum = 32768):
    y[rows_e, :] = x[rows_e, :] @ W[e].T        W[e]: [1408, 2048]

Strategy: pure expert-parallel over 8 NeuronCores — core c owns experts
[8c, 8c+8) and their (contiguous) token rows. No collectives; gather is a
host-side concat. Both operands are cast to fp16 on the host: the PE runs
fp16 at full rate (78.6 TF/s) while halving HBM traffic, which moves the
kernel from DMA-bound (149 MB/core, ~416 us) to compute-bound (~300 us).
y is written back as fp16 too (host upcasts); measured rel err ~3e-4.

Layout: host pre-packs both operands so every DMA is a contiguous run per
partition (128 descriptors, no strided row descriptors — descriptor
generation was ~3.5 us per 1 MB x chunk in the naive [P, KO, T] layout):
  xT[p, KO*t0c + ko*csz + j]  = x[t0c + j, ko*128 + p]   per token-chunk
  wT[p, woff(e,nt,q) + k*nsz + j] = W[e][n0+j, (q*KQ+k)*128 + p]
DMA queues are split per operand (x: sync, W: scalar, y: gpsimd) so the
first matmul's dependencies never serialize behind later traffic, and a
burst of scratch matmuls at kernel start warms the PE HAM clock gate
(1.2 -> 2.4 GHz) during the initial DMA wait.
"""

import math

import numpy as np

import concourse.mybir as mybir
import concourse.tile as tile
from concourse import bacc
from concourse.bass_utils import run_bass_kernel_spmd

N_CORES = 8
P = 128
IN_SIZE = 2048
OUT_SIZE = 1408
KO = IN_SIZE // P  # 16 contraction subtiles
N_TILE = 512
KQ = 4  # W arrives in quarter-K chunks for fine pipelining
NQ = KO // KQ
WARMUP_MM = 16  # scratch matmuls: warm the HAM clock gate AND absorb the
# startup DMA ramp (first x chunk + W quarters land at ~12 us; warmup ends
# then: 4 cold MMs @ ~426 ns + 12 warm @ ~213 ns after the ~7.5 us preamble)

_nc_cache: dict = {}


def _n_tiles():
    tiles = []
    n0 = 0
    while n0 < OUT_SIZE:
        nsz = min(N_TILE, OUT_SIZE - n0)
        tiles.append((n0, nsz))
        n0 += nsz
    return tiles


SEG_MAX = 6 * P  # per-segment token cap: bounds SBUF for arbitrary splits


def _plan_layout(pattern: tuple):
    """Pure function of the (padded) per-expert token pattern; used by both
    _build (device program) and kernel() (host repack) so the DRAM layouts
    agree. Returns segments, processing order, x chunking and W block
    offsets (elements, per partition)."""
    T = sum(pattern)
    segs = []  # (expert, token offset, token count <= SEG_MAX)
    t = 0
    for e, m in enumerate(pattern):
        s0 = 0
        while s0 < m:
            sm = min(SEG_MAX, m - s0)
            segs.append((e, t + s0, sm))
            s0 += sm
        t += m
    chunks = [-(-s[2] // (2 * P)) for s in segs]
    # Fast config (tuned for the alternating small/big pattern): x in
    # 2-m-tile granules, larger segment of each pair first. Requires any
    # adjacent pair to fit in 4 x-slots (bufs=5 leaves one for prefetch);
    # otherwise whole-segment x tiles, which always fit with bufs=2.
    fast = len(segs) > 0 and all(
        chunks[i] + chunks[i + 1] <= 4 for i in range(0, len(segs) - 1, 2)
    )
    if fast:
        XC = 2 * P
        x_bufs = 5
        order = []
        for i in range(0, len(segs) - 1, 2):
            a, b = i, i + 1
            order.extend([b, a] if segs[b][2] > segs[a][2] else [a, b])
        if len(segs) % 2:
            order.append(len(segs) - 1)

    else:
        XC = SEG_MAX
        x_bufs = 2
        order = list(range(len(segs)))

    # x chunks in token order; chunk starting at token t0c occupies
    # xT[:, KO*t0c : KO*(t0c+csz)] (contiguous per partition).
    x_chunks = []  # (t0c, csz)
    seg_chunks = []  # per segment: list of chunk indices
    for e, t0, m in segs:
        idxs = []
        c0 = 0
        while c0 < m:
            csz = min(XC, m - c0)
            idxs.append(len(x_chunks))
            x_chunks.append((t0 + c0, csz))
            c0 += csz
        seg_chunks.append(idxs)

    # W blocks in canonical (expert, n-tile, quarter) order.
    w_offs = {}
    off = 0
    E_loc = len(pattern)
    for e in range(E_loc):
        for nt, (n0, nsz) in enumerate(_n_tiles()):
            for q in range(NQ):
                w_offs[(e, nt, q)] = off
                off += KQ * nsz
    return {
        "T": T,
        "segs": segs,
        "order": order,
        "fast": fast,
        "XC": XC,
        "x_bufs": x_bufs,
        "x_chunks": x_chunks,
        "seg_chunks": seg_chunks,
        "w_offs": w_offs,
        "w_total": off,
    }


def _build(pattern: tuple) -> "bacc.Bacc":
    """One SPMD program: `pattern` = per-expert (padded) token counts for the
    8 local experts of a core; identical across cores."""
    pl = _plan_layout(pattern)
    T, XC = pl["T"], pl["XC"]
    nc = bacc.Bacc(None, target_bir_lowering=False, name="grouped_linear")
    x_dt = mybir.dt.float16
    w_dt = mybir.dt.float16
    y_dt = mybir.dt.float16
    xT = nc.dram_tensor("xT", [P, KO * T], x_dt, kind="ExternalInput")
    wT = nc.dram_tensor("wT", [P, pl["w_total"]], w_dt, kind="ExternalInput")
    y = nc.dram_tensor("y", [T, OUT_SIZE], y_dt, kind="ExternalOutput")

    with tile.TileContext(nc) as tc:
        with (
            tc.tile_pool(name="xp", bufs=pl["x_bufs"]) as xpool,
            tc.tile_pool(name="wp", bufs=10) as wpool,
            tc.tile_pool(name="op", bufs=8) as opool,
            tc.tile_pool(name="ps", bufs=6, space="PSUM") as pspool,
            tc.tile_pool(name="wu", bufs=1) as wupool,
            tc.tile_pool(name="wq", bufs=1, space="PSUM") as wqpool,
        ):
            # HAM warm-up: scratch matmuls with no DMA dependencies run
            # during the initial x/W load, lifting the PE to 2.4 GHz
            # before the first real matmul.
            wu_a = wupool.tile([P, P], x_dt, tag="wua", name="wu_a")
            wu_b = wupool.tile([P, N_TILE], w_dt, tag="wub", name="wu_b")
            wu_ps = wqpool.tile([P, N_TILE], mybir.dt.float32, tag="wups", name="wu_ps")
            nc.vector.memset(wu_a, 0)
            nc.vector.memset(wu_b, 0)
            for _ in range(WARMUP_MM):
                nc.tensor.matmul(wu_ps, wu_a, wu_b, start=True, stop=True)

            for si in pl["order"]:
                e, t0, m = pl["segs"][si]
                mts = m // P
                # All loads go on the sync ring in exact consumption order:
                # x chunk 0, first n-tile's W quarters, remaining x chunks,
                # then the other n-tiles' W. HWDGE rings are FIFO, so this
                # delivers prerequisites just-in-time at full HBM bandwidth.
                x_cs = []

                def _load_chunk(ci):
                    t0c, csz = pl["x_chunks"][ci]
                    x_c = xpool.tile([P, KO * XC], x_dt, tag="x", name="x_c")
                    nc.sync.dma_start(
                        x_c[:, : KO * csz], xT[:, KO * t0c : KO * (t0c + csz)]
                    )
                    x_cs.append((x_c, t0c, csz))

                seg_chunks = pl["seg_chunks"][si]
                _load_chunk(seg_chunks[0])
                for nt, (n0, nsz) in enumerate(_n_tiles()):
                    w_qs = []
                    for q in range(NQ):
                        w_q = wpool.tile([P, KQ * N_TILE], w_dt, tag="w", name="w_q")
                        woff = pl["w_offs"][(e, nt, q)]
                        nc.sync.dma_start(
                            w_q[:, : KQ * nsz], wT[:, woff : woff + KQ * nsz]
                        )
                        w_qs.append(w_q)
                    if nt == 0:
                        for ci in seg_chunks[1:]:
                            _load_chunk(ci)
                    for mt in range(mts):
                        x_c, t0c, csz = x_cs[(mt * P) // XC]
                        xoff = t0 + mt * P - t0c
                        ps_t = pspool.tile(
                            [P, N_TILE], mybir.dt.float32, tag="ps", name="ps_t"
                        )
                        for q in range(NQ):
                            for k in range(KQ):
                                ko = q * KQ + k
                                nc.tensor.matmul(
                                    ps_t[:, :nsz],
                                    x_c[:, ko * csz + xoff : ko * csz + xoff + P],
                                    w_qs[q][:, k * nsz : (k + 1) * nsz],
                                    start=(ko == 0),
                                    stop=(ko == KO - 1),
                                )
                        o_t = opool.tile([P, N_TILE], y_dt, tag="o", name="o_t")
                        nc.vector.tensor_copy(o_t[:, :nsz], ps_t[:, :nsz])
                        nc.scalar.dma_start(
                            y[t0 + mt * P : t0 + (mt + 1) * P, n0 : n0 + nsz],
                            o_t[:, :nsz],
                        )
    nc.compile()
    return nc


def _get_nc(pattern: tuple) -> "bacc.Bacc":
    nc = _nc_cache.get(pattern)
    if nc is None:
        nc = _build(pattern)
        _nc_cache[pattern] = nc
    return nc


def _plan(splits: np.ndarray):
    """Choose a per-core expert-size pattern (identical across cores, sizes
    multiples of 128). Returns (padded_pattern, per-core list of per-expert
    actual sizes)."""
    E = len(splits)
    epc = E // N_CORES
    per_core = [tuple(int(s) for s in splits[c * epc : (c + 1) * epc]) for c in range(N_CORES)]
    uniform = all(p == per_core[0] for p in per_core)
    if uniform:
        padded = tuple(128 * math.ceil(s / 128) for s in per_core[0])
    else:
        m_pad = 128 * math.ceil(int(max(splits.max(), 1)) / 128)
        padded = (m_pad,) * epc
    return padded, per_core


def kernel(x: np.ndarray, W: np.ndarray, m_splits: np.ndarray, _profile=None) -> np.ndarray:
    x = np.ascontiguousarray(np.asarray(x), dtype=np.float32)
    W = np.ascontiguousarray(np.asarray(W), dtype=np.float32)
    raw = np.asarray(m_splits).astype(np.int64)
    E = raw.shape[0]
    assert E % N_CORES == 0 and W.shape[0] == E
    epc = E // N_CORES
    # Mirror the reference's python-slice semantics: x[offs[e]:offs[e+1]]
    # clips to the array bounds, so effective sizes come from clipped offsets.
    raw_offs = np.concatenate([[0], np.cumsum(np.maximum(raw, 0))])
    lo = np.minimum(raw_offs[:-1], x.shape[0])
    hi = np.minimum(raw_offs[1:], x.shape[0])
    splits = np.maximum(hi - lo, 0)
    offs = np.concatenate([[0], np.cumsum(splits)])
    total = int(offs[-1])

    padded, per_core = _plan(splits)
    pofs = np.concatenate([[0], np.cumsum(padded)])
    T_pad = int(pofs[-1])

    nc = _get_nc(padded)
    pl = _plan_layout(padded)

    n_tiles = _n_tiles()
    in_maps = []
    for c in range(N_CORES):
        if tuple(padded) == per_core[c]:
            xs = x[lo[c * epc] : hi[(c + 1) * epc - 1]]
        else:
            xs = np.zeros((T_pad, IN_SIZE), dtype=np.float32)
            for e in range(epc):
                g = c * epc + e
                xs[pofs[e] : pofs[e] + splits[g]] = x[lo[g] : hi[g]]
        xs16 = xs.astype(np.float16)
        xT = np.empty((P, KO * T_pad), dtype=np.float16)
        for t0c, csz in pl["x_chunks"]:
            blk = xs16[t0c : t0c + csz].reshape(csz, KO, P).transpose(2, 1, 0)
            xT[:, KO * t0c : KO * (t0c + csz)] = blk.reshape(P, KO * csz)
        wfull = (
            W[c * epc : (c + 1) * epc]
            .reshape(epc, OUT_SIZE, KO, P)
            .transpose(0, 3, 2, 1)
            .astype(np.float16)
        )  # [epc, P, KO, OUT]
        wT = np.empty((P, pl["w_total"]), dtype=np.float16)
        for (e, nt, q), woff in pl["w_offs"].items():
            n0, nsz = n_tiles[nt]
            blk = wfull[e, :, q * KQ : (q + 1) * KQ, n0 : n0 + nsz]
            wT[:, woff : woff + KQ * nsz] = blk.reshape(P, KQ * nsz)
        in_maps.append({"xT": np.ascontiguousarray(xT), "wT": np.ascontiguousarray(wT)})

    kwargs = dict(_profile) if _profile else {}
    res = run_bass_kernel_spmd(nc, in_maps, core_ids=list(range(N_CORES)), **kwargs)
    if _profile is not None:
        _profile["result"] = res

    out = np.empty((total, OUT_SIZE), dtype=np.float32)
    for c in range(N_CORES):
        yc = res.results[c]["y"]
        for e in range(epc):
            g = c * epc + e
            out[offs[g] : offs[g + 1]] = yc[pofs[e] : pofs[e] + splits[g]]
    return out


# revision 21
# speedup vs baseline: 1.0726x; 1.0025x over previous
"""Trainium2 Bass kernel for MindSpeed TE grouped linear (MoE grouped GEMM).

Computes, for E=64 experts with row splits m_splits (sum = 32768):
    y[rows_e, :] = x[rows_e, :] @ W[e].T        W[e]: [1408, 2048]

Strategy: pure expert-parallel over 8 NeuronCores — core c owns experts
[8c, 8c+8) and their (contiguous) token rows. No collectives; gather is a
host-side concat. Both operands are cast to fp16 on the host: the PE runs
fp16 at full rate (78.6 TF/s) while halving HBM traffic, which moves the
kernel from DMA-bound (149 MB/core, ~416 us) to compute-bound (~300 us).
y is written back as fp16 too (host upcasts); measured rel err ~3e-4.

Layout: host pre-packs both operands so every DMA is a contiguous run per
partition (128 descriptors, no strided row descriptors — descriptor
generation was ~3.5 us per 1 MB x chunk in the naive [P, KO, T] layout):
  xT[p, KO*t0c + ko*csz + j]  = x[t0c + j, ko*128 + p]   per token-chunk
  wT[p, woff(e,nt,q) + k*nsz + j] = W[e][n0+j, (q*KQ+k)*128 + p]
All loads (x chunks + W quarters) go on the sync HWDGE ring in exact
consumption order — FIFO rings deliver prerequisites just-in-time at full
HBM bandwidth, which beats splitting loads across rings (each then gets
only a share of the 358 GB/s exactly when the ramp needs all of it).
Stores go on the scalar HWDGE ring (never gpsimd/SWDGE: its end-of-kernel
Q7 drain costs ~9 us). A burst of scratch matmuls at kernel start warms
the PE HAM clock gate (1.2 -> 2.4 GHz) and absorbs the initial DMA ramp.
"""

import math

import numpy as np

import concourse.mybir as mybir
import concourse.tile as tile
from concourse import bacc
from concourse.bass_utils import run_bass_kernel_spmd

N_CORES = 8
P = 128
IN_SIZE = 2048
OUT_SIZE = 1408
KO = IN_SIZE // P  # 16 contraction subtiles
N_TILE = 512
KQ = 4  # W arrives in quarter-K chunks for fine pipelining
NQ = KO // KQ
WARMUP_MM = 18  # scratch matmuls: warm the HAM clock gate AND absorb the
# startup DMA ramp (first x chunk + W quarters land at ~12 us; warmup ends
# then: 4 cold MMs @ ~426 ns + 12 warm @ ~213 ns after the ~7.5 us preamble)

_nc_cache: dict = {}


def _n_tiles():
    tiles = []
    n0 = 0
    while n0 < OUT_SIZE:
        nsz = min(N_TILE, OUT_SIZE - n0)
        tiles.append((n0, nsz))
        n0 += nsz
    return tiles


SEG_MAX = 6 * P  # per-segment token cap: bounds SBUF for arbitrary splits


def _plan_layout(pattern: tuple):
    """Pure function of the (padded) per-expert token pattern; used by both
    _build (device program) and kernel() (host repack) so the DRAM layouts
    agree. Returns segments, processing order, x chunking and W block
    offsets (elements, per partition)."""
    T = sum(pattern)
    segs = []  # (expert, token offset, token count <= SEG_MAX)
    t = 0
    for e, m in enumerate(pattern):
        s0 = 0
        while s0 < m:
            sm = min(SEG_MAX, m - s0)
            segs.append((e, t + s0, sm))
            s0 += sm
        t += m
    chunks = [-(-s[2] // (2 * P)) for s in segs]
    # Fast config (tuned for the alternating small/big pattern): x in
    # 2-m-tile granules, larger segment of each pair first. Requires any
    # adjacent pair to fit in 4 x-slots (bufs=5 leaves one for prefetch);
    # otherwise whole-segment x tiles, which always fit with bufs=2.
    fast = len(segs) > 0 and all(
        chunks[i] + chunks[i + 1] <= 4 for i in range(0, len(segs) - 1, 2)
    )
    if fast:
        XC = 2 * P
        x_bufs = 6
        order = []
        for i in range(0, len(segs) - 1, 2):
            a, b = i, i + 1
            order.extend([b, a] if segs[b][2] > segs[a][2] else [a, b])
        if len(segs) % 2:
            order.append(len(segs) - 1)

    else:
        XC = SEG_MAX
        x_bufs = 2
        order = list(range(len(segs)))

    # x chunks in token order; chunk starting at token t0c occupies
    # xT[:, KO*t0c : KO*(t0c+csz)] (contiguous per partition).
    x_chunks = []  # (t0c, csz)
    seg_chunks = []  # per segment: list of chunk indices
    for e, t0, m in segs:
        idxs = []
        c0 = 0
        while c0 < m:
            csz = min(XC, m - c0)
            idxs.append(len(x_chunks))
            x_chunks.append((t0 + c0, csz))
            c0 += csz
        seg_chunks.append(idxs)

    # W blocks in canonical (expert, n-tile, quarter) order.
    w_offs = {}
    off = 0
    E_loc = len(pattern)
    for e in range(E_loc):
        for nt, (n0, nsz) in enumerate(_n_tiles()):
            for q in range(NQ):
                w_offs[(e, nt, q)] = off
                off += KQ * nsz
    return {
        "T": T,
        "segs": segs,
        "order": order,
        "fast": fast,
        "XC": XC,
        "x_bufs": x_bufs,
        "x_chunks": x_chunks,
        "seg_chunks": seg_chunks,
        "w_offs": w_offs,
        "w_total": off,
    }


def _build(pattern: tuple) -> "bacc.Bacc":
    """One SPMD program: `pattern` = per-expert (padded) token counts for the
    8 local experts of a core; identical across cores."""
    pl = _plan_layout(pattern)
    T, XC = pl["T"], pl["XC"]
    nc = bacc.Bacc(None, target_bir_lowering=False, name="grouped_linear")
    x_dt = mybir.dt.float16
    w_dt = mybir.dt.float16
    y_dt = mybir.dt.float16
    xT = nc.dram_tensor("xT", [P, KO * T], x_dt, kind="ExternalInput")
    wT = nc.dram_tensor("wT", [P, pl["w_total"]], w_dt, kind="ExternalInput")
    y = nc.dram_tensor("y", [T, OUT_SIZE], y_dt, kind="ExternalOutput")

    with tile.TileContext(nc) as tc:
        with (
            tc.tile_pool(name="xp", bufs=pl["x_bufs"]) as xpool,
            tc.tile_pool(name="wp", bufs=16) as wpool,
            tc.tile_pool(name="op", bufs=8) as opool,
            tc.tile_pool(name="ps", bufs=6, space="PSUM") as pspool,
            tc.tile_pool(name="wu", bufs=1) as wupool,
            tc.tile_pool(name="wq", bufs=1, space="PSUM") as wqpool,
        ):
            # HAM warm-up: scratch matmuls with no DMA dependencies run
            # during the initial x/W load, lifting the PE to 2.4 GHz
            # before the first real matmul.
            wu_a = wupool.tile([P, P], x_dt, tag="wua", name="wu_a")
            wu_b = wupool.tile([P, N_TILE], w_dt, tag="wub", name="wu_b")
            wu_ps = wqpool.tile([P, N_TILE], mybir.dt.float32, tag="wups", name="wu_ps")
            nc.vector.memset(wu_a, 0)
            nc.vector.memset(wu_b, 0)
            for _ in range(WARMUP_MM):
                nc.tensor.matmul(wu_ps, wu_a, wu_b, start=True, stop=True)

            for si in pl["order"]:
                e, t0, m = pl["segs"][si]
                mts = m // P
                # All loads go on the sync ring in exact consumption order:
                # x chunk 0, first n-tile's W quarters, remaining x chunks,
                # then the other n-tiles' W. HWDGE rings are FIFO, so this
                # delivers prerequisites just-in-time at full HBM bandwidth.
                x_cs = []

                def _load_chunk(ci):
                    t0c, csz = pl["x_chunks"][ci]
                    x_c = xpool.tile([P, KO * XC], x_dt, tag="x", name="x_c")
                    nc.sync.dma_start(
                        x_c[:, : KO * csz], xT[:, KO * t0c : KO * (t0c + csz)]
                    )
                    x_cs.append((x_c, t0c, csz))

                seg_chunks = pl["seg_chunks"][si]
                _load_chunk(seg_chunks[0])
                for nt, (n0, nsz) in enumerate(_n_tiles()):
                    w_qs = []
                    for q in range(NQ):
                        w_q = wpool.tile([P, KQ * N_TILE], w_dt, tag="w", name="w_q")
                        woff = pl["w_offs"][(e, nt, q)]
                        nc.sync.dma_start(
                            w_q[:, : KQ * nsz], wT[:, woff : woff + KQ * nsz]
                        )
                        w_qs.append(w_q)
                    if nt == 0:
                        for ci in seg_chunks[1:]:
                            _load_chunk(ci)
                    for mt in range(mts):
                        x_c, t0c, csz = x_cs[(mt * P) // XC]
                        xoff = t0 + mt * P - t0c
                        ps_t = pspool.tile(
                            [P, N_TILE], mybir.dt.float32, tag="ps", name="ps_t"
                        )
                        for q in range(NQ):
                            for k in range(KQ):
                                ko = q * KQ + k
                                nc.tensor.matmul(
                                    ps_t[:, :nsz],
                                    x_c[:, ko * csz + xoff : ko * csz + xoff + P],
                                    w_qs[q][:, k * nsz : (k + 1) * nsz],
                                    start=(ko == 0),
                                    stop=(ko == KO - 1),
                                )
                        o_t = opool.tile([P, N_TILE], y_dt, tag="o", name="o_t")
                        nc.vector.tensor_copy(o_t[:, :nsz], ps_t[:, :nsz])
                        nc.scalar.dma_start(
                            y[t0 + mt * P : t0 + (mt + 1) * P, n0 : n0 + nsz],
                            o_t[:, :nsz],
                        )
    nc.compile()
    return nc


def _get_nc(pattern: tuple) -> "bacc.Bacc":
    nc = _nc_cache.get(pattern)
    if nc is None:
        nc = _build(pattern)
        _nc_cache[pattern] = nc
    return nc


def _plan(splits: np.ndarray):
    """Choose a per-core expert-size pattern (identical across cores, sizes
    multiples of 128). Returns (padded_pattern, per-core list of per-expert
    actual sizes)."""
    E = len(splits)
    epc = E // N_CORES
    per_core = [tuple(int(s) for s in splits[c * epc : (c + 1) * epc]) for c in range(N_CORES)]
    uniform = all(p == per_core[0] for p in per_core)
    if uniform:
        padded = tuple(128 * math.ceil(s / 128) for s in per_core[0])
    else:
        m_pad = 128 * math.ceil(int(max(splits.max(), 1)) / 128)
        padded = (m_pad,) * epc
    return padded, per_core


def kernel(x: np.ndarray, W: np.ndarray, m_splits: np.ndarray, _profile=None) -> np.ndarray:
    x = np.ascontiguousarray(np.asarray(x), dtype=np.float32)
    W = np.ascontiguousarray(np.asarray(W), dtype=np.float32)
    raw = np.asarray(m_splits).astype(np.int64)
    E = raw.shape[0]
    assert E % N_CORES == 0 and W.shape[0] == E
    epc = E // N_CORES
    # Mirror the reference's python-slice semantics: x[offs[e]:offs[e+1]]
    # clips to the array bounds, so effective sizes come from clipped offsets.
    raw_offs = np.concatenate([[0], np.cumsum(np.maximum(raw, 0))])
    lo = np.minimum(raw_offs[:-1], x.shape[0])
    hi = np.minimum(raw_offs[1:], x.shape[0])
    splits = np.maximum(hi - lo, 0)
    offs = np.concatenate([[0], np.cumsum(splits)])
    total = int(offs[-1])

    padded, per_core = _plan(splits)
    pofs = np.concatenate([[0], np.cumsum(padded)])
    T_pad = int(pofs[-1])

    nc = _get_nc(padded)
    pl = _plan_layout(padded)

    n_tiles = _n_tiles()
    in_maps = []
    for c in range(N_CORES):
        if tuple(padded) == per_core[c]:
            xs = x[lo[c * epc] : hi[(c + 1) * epc - 1]]
        else:
            xs = np.zeros((T_pad, IN_SIZE), dtype=np.float32)
            for e in range(epc):
                g = c * epc + e
                xs[pofs[e] : pofs[e] + splits[g]] = x[lo[g] : hi[g]]
        xs16 = xs.astype(np.float16)
        xT = np.empty((P, KO * T_pad), dtype=np.float16)
        for t0c, csz in pl["x_chunks"]:
            blk = xs16[t0c : t0c + csz].reshape(csz, KO, P).transpose(2, 1, 0)
            xT[:, KO * t0c : KO * (t0c + csz)] = blk.reshape(P, KO * csz)
        wfull = (
            W[c * epc : (c + 1) * epc]
            .reshape(epc, OUT_SIZE, KO, P)
            .transpose(0, 3, 2, 1)
            .astype(np.float16)
        )  # [epc, P, KO, OUT]
        wT = np.empty((P, pl["w_total"]), dtype=np.float16)
        for (e, nt, q), woff in pl["w_offs"].items():
            n0, nsz = n_tiles[nt]
            blk = wfull[e, :, q * KQ : (q + 1) * KQ, n0 : n0 + nsz]
            wT[:, woff : woff + KQ * nsz] = blk.reshape(P, KQ * nsz)
        in_maps.append({"xT": np.ascontiguousarray(xT), "wT": np.ascontiguousarray(wT)})

    kwargs = dict(_profile) if _profile else {}
    res = run_bass_kernel_spmd(nc, in_maps, core_ids=list(range(N_CORES)), **kwargs)
    if _profile is not None:
        _profile["result"] = res

    out = np.empty((total, OUT_SIZE), dtype=np.float32)
    for c in range(N_CORES):
        yc = res.results[c]["y"]
        for e in range(epc):
            g = c * epc + e
            out[offs[g] : offs[g + 1]] = yc[pofs[e] : pofs[e] + splits[g]]
    return out


# revision 22
# speedup vs baseline: 1.0730x; 1.0004x over previous
"""Trainium2 Bass kernel for MindSpeed TE grouped linear (MoE grouped GEMM).

Computes, for E=64 experts with row splits m_splits (sum = 32768):
    y[rows_e, :] = x[rows_e, :] @ W[e].T        W[e]: [1408, 2048]

Strategy: pure expert-parallel over 8 NeuronCores — core c owns experts
[8c, 8c+8) and their (contiguous) token rows. No collectives; gather is a
host-side concat. Both operands are cast to fp16 on the host: the PE runs
fp16 at full rate (78.6 TF/s) while halving HBM traffic, which moves the
kernel from DMA-bound (149 MB/core, ~416 us) to compute-bound (~300 us).
y is written back as fp16 too (host upcasts); measured rel err ~3e-4.

Layout: host pre-packs both operands so every DMA is a contiguous run per
partition (128 descriptors, no strided row descriptors — descriptor
generation was ~3.5 us per 1 MB x chunk in the naive [P, KO, T] layout):
  xT[p, KO*t0c + ko*csz + j]  = x[t0c + j, ko*128 + p]   per token-chunk
  wT[p, woff(e,nt,q) + k*nsz + j] = W[e][n0+j, (q*KQ+k)*128 + p]
All loads (x chunks + W quarters) go on the sync HWDGE ring in exact
consumption order — FIFO rings deliver prerequisites just-in-time at full
HBM bandwidth, which beats splitting loads across rings (each then gets
only a share of the 358 GB/s exactly when the ramp needs all of it).
Stores go on the scalar HWDGE ring (never gpsimd/SWDGE: its end-of-kernel
Q7 drain costs ~9 us). A burst of scratch matmuls at kernel start warms
the PE HAM clock gate (1.2 -> 2.4 GHz) and absorbs the initial DMA ramp.
"""

import math

import numpy as np

import concourse.mybir as mybir
import concourse.tile as tile
from concourse import bacc
from concourse.bass_utils import run_bass_kernel_spmd

N_CORES = 8
P = 128
IN_SIZE = 2048
OUT_SIZE = 1408
KO = IN_SIZE // P  # 16 contraction subtiles
N_TILE = 512
KQ = 4  # W arrives in quarter-K chunks for fine pipelining
NQ = KO // KQ
WARMUP_MM = 14  # scratch matmuls: warm the HAM clock gate AND absorb the
# startup DMA ramp (first x chunk + W quarters land at ~12 us; warmup ends
# then: 4 cold MMs @ ~426 ns + 12 warm @ ~213 ns after the ~7.5 us preamble)

_nc_cache: dict = {}


def _n_tiles():
    tiles = []
    n0 = 0
    while n0 < OUT_SIZE:
        nsz = min(N_TILE, OUT_SIZE - n0)
        tiles.append((n0, nsz))
        n0 += nsz
    return tiles


SEG_MAX = 6 * P  # per-segment token cap: bounds SBUF for arbitrary splits


def _plan_layout(pattern: tuple):
    """Pure function of the (padded) per-expert token pattern; used by both
    _build (device program) and kernel() (host repack) so the DRAM layouts
    agree. Returns segments, processing order, x chunking and W block
    offsets (elements, per partition)."""
    T = sum(pattern)
    segs = []  # (expert, token offset, token count <= SEG_MAX)
    t = 0
    for e, m in enumerate(pattern):
        s0 = 0
        while s0 < m:
            sm = min(SEG_MAX, m - s0)
            segs.append((e, t + s0, sm))
            s0 += sm
        t += m
    chunks = [-(-s[2] // (2 * P)) for s in segs]
    # Fast config (tuned for the alternating small/big pattern): x in
    # 2-m-tile granules, larger segment of each pair first. Requires any
    # adjacent pair to fit in 4 x-slots (bufs=5 leaves one for prefetch);
    # otherwise whole-segment x tiles, which always fit with bufs=2.
    fast = len(segs) > 0 and all(
        chunks[i] + chunks[i + 1] <= 4 for i in range(0, len(segs) - 1, 2)
    )
    if fast:
        XC = 2 * P
        x_bufs = 6
        order = []
        for i in range(0, len(segs) - 1, 2):
            a, b = i, i + 1
            order.extend([b, a] if segs[b][2] > segs[a][2] else [a, b])
        if len(segs) % 2:
            order.append(len(segs) - 1)

    else:
        XC = SEG_MAX
        x_bufs = 2
        order = list(range(len(segs)))

    # x chunks in token order; chunk starting at token t0c occupies
    # xT[:, KO*t0c : KO*(t0c+csz)] (contiguous per partition).
    x_chunks = []  # (t0c, csz)
    seg_chunks = []  # per segment: list of chunk indices
    for e, t0, m in segs:
        idxs = []
        c0 = 0
        while c0 < m:
            csz = min(XC, m - c0)
            idxs.append(len(x_chunks))
            x_chunks.append((t0 + c0, csz))
            c0 += csz
        seg_chunks.append(idxs)

    # W blocks in canonical (expert, n-tile, quarter) order.
    w_offs = {}
    off = 0
    E_loc = len(pattern)
    for e in range(E_loc):
        for nt, (n0, nsz) in enumerate(_n_tiles()):
            for q in range(NQ):
                w_offs[(e, nt, q)] = off
                off += KQ * nsz
    return {
        "T": T,
        "segs": segs,
        "order": order,
        "fast": fast,
        "XC": XC,
        "x_bufs": x_bufs,
        "x_chunks": x_chunks,
        "seg_chunks": seg_chunks,
        "w_offs": w_offs,
        "w_total": off,
    }


def _build(pattern: tuple) -> "bacc.Bacc":
    """One SPMD program: `pattern` = per-expert (padded) token counts for the
    8 local experts of a core; identical across cores."""
    pl = _plan_layout(pattern)
    T, XC = pl["T"], pl["XC"]
    nc = bacc.Bacc(None, target_bir_lowering=False, name="grouped_linear")
    x_dt = mybir.dt.float16
    w_dt = mybir.dt.float16
    y_dt = mybir.dt.float16
    xT = nc.dram_tensor("xT", [P, KO * T], x_dt, kind="ExternalInput")
    wT = nc.dram_tensor("wT", [P, pl["w_total"]], w_dt, kind="ExternalInput")
    y = nc.dram_tensor("y", [T, OUT_SIZE], y_dt, kind="ExternalOutput")

    with tile.TileContext(nc) as tc:
        with (
            tc.tile_pool(name="xp", bufs=pl["x_bufs"]) as xpool,
            tc.tile_pool(name="wp", bufs=13) as wpool,
            tc.tile_pool(name="op", bufs=8) as opool,
            tc.tile_pool(name="ps", bufs=7, space="PSUM") as pspool,
            tc.tile_pool(name="wu", bufs=1) as wupool,
            tc.tile_pool(name="wq", bufs=1, space="PSUM") as wqpool,
        ):
            # HAM warm-up: scratch matmuls with no DMA dependencies run
            # during the initial x/W load, lifting the PE to 2.4 GHz
            # before the first real matmul.
            wu_a = wupool.tile([P, P], x_dt, tag="wua", name="wu_a")
            wu_b = wupool.tile([P, N_TILE], w_dt, tag="wub", name="wu_b")
            wu_ps = wqpool.tile([P, N_TILE], mybir.dt.float32, tag="wups", name="wu_ps")
            nc.vector.memset(wu_a, 0)
            nc.vector.memset(wu_b, 0)
            for _ in range(WARMUP_MM):
                nc.tensor.matmul(wu_ps, wu_a, wu_b, start=True, stop=True)

            for si in pl["order"]:
                e, t0, m = pl["segs"][si]
                mts = m // P
                # All loads go on the sync ring in exact consumption order:
                # x chunk 0, first n-tile's W quarters, remaining x chunks,
                # then the other n-tiles' W. HWDGE rings are FIFO, so this
                # delivers prerequisites just-in-time at full HBM bandwidth.
                x_cs = []

                def _load_chunk(ci):
                    t0c, csz = pl["x_chunks"][ci]
                    x_c = xpool.tile([P, KO * XC], x_dt, tag="x", name="x_c")
                    nc.sync.dma_start(
                        x_c[:, : KO * csz], xT[:, KO * t0c : KO * (t0c + csz)]
                    )
                    x_cs.append((x_c, t0c, csz))

                seg_chunks = pl["seg_chunks"][si]
                _load_chunk(seg_chunks[0])
                for nt, (n0, nsz) in enumerate(_n_tiles()):
                    w_qs = []
                    for q in range(NQ):
                        w_q = wpool.tile([P, KQ * N_TILE], w_dt, tag="w", name="w_q")
                        woff = pl["w_offs"][(e, nt, q)]
                        nc.sync.dma_start(
                            w_q[:, : KQ * nsz], wT[:, woff : woff + KQ * nsz]
                        )
                        w_qs.append(w_q)
                    if nt == 0:
                        for ci in seg_chunks[1:]:
                            _load_chunk(ci)
                    for mt in range(mts):
                        x_c, t0c, csz = x_cs[(mt * P) // XC]
                        xoff = t0 + mt * P - t0c
                        ps_t = pspool.tile(
                            [P, N_TILE], mybir.dt.float32, tag="ps", name="ps_t"
                        )
                        for q in range(NQ):
                            for k in range(KQ):
                                ko = q * KQ + k
                                nc.tensor.matmul(
                                    ps_t[:, :nsz],
                                    x_c[:, ko * csz + xoff : ko * csz + xoff + P],
                                    w_qs[q][:, k * nsz : (k + 1) * nsz],
                                    start=(ko == 0),
                                    stop=(ko == KO - 1),
                                )
                        o_t = opool.tile([P, N_TILE], y_dt, tag="o", name="o_t")
                        nc.vector.tensor_copy(o_t[:, :nsz], ps_t[:, :nsz])
                        nc.scalar.dma_start(
                            y[t0 + mt * P : t0 + (mt + 1) * P, n0 : n0 + nsz],
                            o_t[:, :nsz],
                        )
    nc.compile()
    return nc


def _get_nc(pattern: tuple) -> "bacc.Bacc":
    nc = _nc_cache.get(pattern)
    if nc is None:
        nc = _build(pattern)
        _nc_cache[pattern] = nc
    return nc


def _plan(splits: np.ndarray):
    """Choose a per-core expert-size pattern (identical across cores, sizes
    multiples of 128). Returns (padded_pattern, per-core list of per-expert
    actual sizes)."""
    E = len(splits)
    epc = E // N_CORES
    per_core = [tuple(int(s) for s in splits[c * epc : (c + 1) * epc]) for c in range(N_CORES)]
    uniform = all(p == per_core[0] for p in per_core)
    if uniform:
        padded = tuple(128 * math.ceil(s / 128) for s in per_core[0])
    else:
        m_pad = 128 * math.ceil(int(max(splits.max(), 1)) / 128)
        padded = (m_pad,) * epc
    return padded, per_core


def kernel(x: np.ndarray, W: np.ndarray, m_splits: np.ndarray, _profile=None) -> np.ndarray:
    x = np.ascontiguousarray(np.asarray(x), dtype=np.float32)
    W = np.ascontiguousarray(np.asarray(W), dtype=np.float32)
    raw = np.asarray(m_splits).astype(np.int64)
    E = raw.shape[0]
    assert E % N_CORES == 0 and W.shape[0] == E
    epc = E // N_CORES
    # Mirror the reference's python-slice semantics: x[offs[e]:offs[e+1]]
    # clips to the array bounds, so effective sizes come from clipped offsets.
    raw_offs = np.concatenate([[0], np.cumsum(np.maximum(raw, 0))])
    lo = np.minimum(raw_offs[:-1], x.shape[0])
    hi = np.minimum(raw_offs[1:], x.shape[0])
    splits = np.maximum(hi - lo, 0)
    offs = np.concatenate([[0], np.cumsum(splits)])
    total = int(offs[-1])

    padded, per_core = _plan(splits)
    pofs = np.concatenate([[0], np.cumsum(padded)])
    T_pad = int(pofs[-1])

    nc = _get_nc(padded)
    pl = _plan_layout(padded)

    n_tiles = _n_tiles()
    in_maps = []
    for c in range(N_CORES):
        if tuple(padded) == per_core[c]:
            xs = x[lo[c * epc] : hi[(c + 1) * epc - 1]]
        else:
            xs = np.zeros((T_pad, IN_SIZE), dtype=np.float32)
            for e in range(epc):
                g = c * epc + e
                xs[pofs[e] : pofs[e] + splits[g]] = x[lo[g] : hi[g]]
        xs16 = xs.astype(np.float16)
        xT = np.empty((P, KO * T_pad), dtype=np.float16)
        for t0c, csz in pl["x_chunks"]:
            blk = xs16[t0c : t0c + csz].reshape(csz, KO, P).transpose(2, 1, 0)
            xT[:, KO * t0c : KO * (t0c + csz)] = blk.reshape(P, KO * csz)
        wfull = (
            W[c * epc : (c + 1) * epc]
            .reshape(epc, OUT_SIZE, KO, P)
            .transpose(0, 3, 2, 1)
            .astype(np.float16)
        )  # [epc, P, KO, OUT]
        wT = np.empty((P, pl["w_total"]), dtype=np.float16)
        for (e, nt, q), woff in pl["w_offs"].items():
            n0, nsz = n_tiles[nt]
            blk = wfull[e, :, q * KQ : (q + 1) * KQ, n0 : n0 + nsz]
            wT[:, woff : woff + KQ * nsz] = blk.reshape(P, KQ * nsz)
        in_maps.append({"xT": np.ascontiguousarray(xT), "wT": np.ascontiguousarray(wT)})

    kwargs = dict(_profile) if _profile else {}
    res = run_bass_kernel_spmd(nc, in_maps, core_ids=list(range(N_CORES)), **kwargs)
    if _profile is not None:
        _profile["result"] = res

    out = np.empty((total, OUT_SIZE), dtype=np.float32)
    for c in range(N_CORES):
        yc = res.results[c]["y"]
        for e in range(epc):
            g = c * epc + e
            out[offs[g] : offs[g + 1]] = yc[pofs[e] : pofs[e] + splits[g]]
    return out
